# revision 12
# baseline (speedup 1.0000x reference)
"""PointWarping2 (Gaussian-kernel Nadaraya-Watson flow regression) on 8 TRN2 cores.

Math (per batch b):
    y      = xyz1 + flow1                     # warped sources  [N1, 3]
    d2     = ||x2_n - y_m||^2                 # [N2, N1]
    K      = exp(-d2 / scale^2)
    flow2  = (K @ [f1|1]) ratios              # Nadaraya-Watson
    out    = x2 - flow2                       # [3, N2]

Device strategy (per core; 8 cores = 2 batches x 4 query-chunks of 2048):
    T[m, n] = exp(-d2[n, m]) computed in "source-major" layout so the second
    matmul (contraction over sources) consumes T tiles directly.

    Loop: j over 512-query chunks, g over "quads" of 4 source tiles (512 rows).
      mm1 (x4, row-packed via tile_position=(32u, 0), K=5):
           S[:, 512u:512u+512] = Yrep[32u:32u+5, quad g].T @ Qrep[32u:32u+5, j]
           with Y rows [y0, y1, y2, |y|^2, 1], Q rows [-2x, 1, |x|^2]
           => S = d2 exactly (f32 PSUM, quad = 4 banks, double buffered)
      ACT: t[128, 2048] = exp(-S / scale^2)   (bf16, SBUF) — one call per quad
      mm2 (x4, K=128, M=97): partial[97, 512] += Vexp[:, tile].T @ t-slice
           accumulated across the quad's 4 tiles into the (now dead) first
           bank of this quad's own S tile; Vexp columns [f0@0, f1@32, f2@64,
           1@96] put num/den rows on 32-aligned partitions (compute APs may
           only start at partitions 0/32/64/96).
      DVE: acc_sb[97, 512] (+)= partial — SBUF accumulator across quads.
    epilogue per j (DVE): flow2 = num * recip(den); out = x2 - flow2.
"""

import os
import sys

import numpy as np

sys.path.insert(0, "/opt/trn_rl_repo")

import ml_dtypes

import concourse.bass as bass
import concourse.mybir as mybir
import concourse.tile as tile
from concourse import bacc
from concourse.bass_utils import run_bass_kernel_spmd

B, C, N1, N2 = 2, 3, 8192, 8192
INITIAL_RADIUS = 1.0
N_CORES = 8
CHUNK = N2 // 4          # queries per core (2 batches x 4 chunks)
JW = 512                 # n2 width per pass
NT1 = N1 // 128          # 64 source tiles of 128
NQUAD = NT1 // 4         # 16 quads of 4 source tiles

LAST_RESULTS = None      # BassKernelResults of the most recent run (for test.py)


def _install_ntff_shim():
    """Register the axon NTFF profiling hook under antenv.axon_hooks (the
    agent image's antenv lacks that submodule) so run_bass_kernel_spmd's
    trace=True path can capture real HW timing. Trace-mode only."""
    import types

    import antenv

    if "antenv.axon_hooks" in sys.modules:
        return
    from trn_agent_boot.trn_boot import _ntff_profile_via_ctypes

    hook = _ntff_profile_via_ctypes("/opt/axon/libaxon_pjrt.so")
    mod = types.ModuleType("antenv.axon_hooks")
    mod._hook = hook
    mod.get_axon_ntff_profile_hook = lambda: mod._hook
    mod.set_axon_ntff_profile_hook = lambda h: setattr(mod, "_hook", h)
    sys.modules["antenv.axon_hooks"] = mod
    antenv.axon_hooks = mod

    # No S3 in this container: stub the artifact upload the trace path does.
    import concourse.bass_utils as bu

    bu.upload_artifacts = lambda tmpdir: tmpdir


def _build_nc(inv_scale2: float) -> bass.Bass:
    nc = bacc.Bacc("TRN2", target_bir_lowering=False, debug=False)
    bf16 = mybir.dt.bfloat16
    f32 = mybir.dt.float32

    qt_d = nc.dram_tensor("qt", [128, CHUNK], bf16, kind="ExternalInput")
    yt_d = nc.dram_tensor("yt", [128, 128 * NQUAD], bf16, kind="ExternalInput")
    vt_d = nc.dram_tensor("vt", [128, 4 * NT1], bf16, kind="ExternalInput")
    x2_d = nc.dram_tensor("x2", [3, CHUNK], f32, kind="ExternalInput")
    out_d = nc.dram_tensor("out", [3, CHUNK], f32, kind="ExternalOutput")

    with tile.TileContext(nc) as tc:
        with (
            tc.tile_pool(name="const", bufs=1) as cpool,
            tc.tile_pool(name="work", bufs=3) as wpool,
            tc.tile_pool(name="spsum", bufs=2, space="PSUM") as spool,
        ):
            qt = cpool.tile([128, CHUNK], bf16)
            yt = cpool.tile([128, 128 * NQUAD], bf16)
            vt = cpool.tile([128, 4 * NT1], bf16)
            # x2 rows as separate base-0 tiles: compute-engine APs may only
            # start at partition 0/32/64/96, so [3, N] row slices are illegal.
            x2r = [cpool.tile([1, CHUNK], f32, tag=f"x2_{c}", name=f"x2_{c}") for c in range(3)]
            nc.sync.dma_start(qt[:], qt_d[:])
            nc.sync.dma_start(yt[:], yt_d[:])
            nc.sync.dma_start(vt[:], vt_d[:])
            for c in range(3):
                nc.sync.dma_start(x2r[c][:], x2_d[c:c + 1, :])
            # Expand V to [128, 97*NT1]: per source tile, column 32*c carries
            # component c (f0/f1/f2/ones), zeros elsewhere, so the mm2 output
            # rows land on partitions 0/32/64/96.
            vexp = cpool.tile([128, 97 * NT1], bf16)
            nc.vector.memset(vexp[:], 0.0)
            for c in range(4):
                nc.vector.tensor_copy(vexp[:, 32 * c::97], vt[:, c::4])

            for j in range(CHUNK // JW):
                js = slice(j * JW, (j + 1) * JW)
                acc = wpool.tile([97, JW], f32, tag="accsb")
                prev = None

                def emit_mm2(prev_s, prev_t, prev_g, acc=acc):
                    for u in range(4):
                        m = 4 * prev_g + u
                        nc.tensor.matmul(
                            prev_s[0:97, 0:JW],
                            vexp[:, 97 * m:97 * m + 97],
                            prev_t[:, u * JW:(u + 1) * JW],
                            start=(u == 0),
                            stop=(u == 3),
                        )
                    if prev_g == 0:
                        nc.vector.tensor_copy(acc[:], prev_s[0:97, 0:JW])
                    else:
                        nc.vector.tensor_add(acc[:], acc[:], prev_s[0:97, 0:JW])

                for g in range(NQUAD):
                    s = spool.tile([128, 4 * JW], f32, tag="s")
                    for u in range(4):
                        nc.tensor.matmul(
                            s[:, u * JW:(u + 1) * JW],
                            yt[32 * u:32 * u + 5, 128 * g:128 * (g + 1)],
                            qt[32 * u:32 * u + 5, js],
                            start=True,
                            stop=True,
                            tile_position=(32 * u, 0),
                        )
                    # mm2 of the previous quad is emitted after this quad's
                    # mm1 so the in-order PE queue never blocks mm1(g+1)
                    # behind mm2(g)'s wait on ACT(g).
                    if prev is not None:
                        emit_mm2(*prev)
                    t = wpool.tile([128, 4 * JW], bf16, tag="t")
                    nc.scalar.activation(
                        t[:],
                        s[:],
                        mybir.ActivationFunctionType.Exp,
                        scale=-float(inv_scale2),
                    )
                    prev = (s, t, g)
                emit_mm2(*prev)

                # Epilogue on DVE: rows live at 32-aligned partitions.
                rec = wpool.tile([1, JW], f32, tag="rec")
                nc.vector.reciprocal(rec[:], acc[96:97, :])
                for c in range(3):
                    ob = wpool.tile([1, JW], f32, tag=f"ob_{c}", name=f"ob_{c}")
                    # two-input DVE ops need equal base partitions in SBUF:
                    # stage the row at base 0 first.
                    nc.vector.tensor_copy(ob[:], acc[32 * c:32 * c + 1, :])
                    nc.vector.tensor_mul(ob[:], ob[:], rec[:])
                    nc.vector.tensor_sub(ob[:], x2r[c][:, js], ob[:])
                    nc.sync.dma_start(out_d[c:c + 1, js], ob[:])

    nc.compile()
    return nc


def kernel(xyz1, xyz2, flow1, resol_factor):
    global LAST_RESULTS
    xyz1 = np.asarray(xyz1, dtype=np.float32)
    xyz2 = np.asarray(xyz2, dtype=np.float32)
    flow1 = np.asarray(flow1, dtype=np.float32)
    scale = INITIAL_RADIUS * float(np.asarray(resol_factor))
    inv_scale2 = 1.0 / (scale * scale)

    bf16 = ml_dtypes.bfloat16

    # Host-side O(N) prep of the augmented operand layouts.
    y = xyz1 + flow1                                  # [B, 3, N1]
    ytil = np.empty((B, 5, N1), np.float32)
    ytil[:, 0:3] = y
    ytil[:, 3] = np.sum(y * y, axis=1)
    ytil[:, 4] = 1.0
    qtil = np.empty((B, 5, N2), np.float32)
    qtil[:, 0:3] = -2.0 * xyz2
    qtil[:, 3] = 1.0
    qtil[:, 4] = np.sum(xyz2 * xyz2, axis=1)

    # Row-replicated layouts for tile_position row-packing: strip u (partition
    # base 32u) of quad column g holds source tile 4g+u; queries replicated
    # on all four strips.
    ytq = ytil.reshape(B, 5, NQUAD, 4, 128)           # [B, r, g, u, p]
    yrep = np.zeros((B, 128, 128 * NQUAD), np.float32)
    qrep = np.zeros((B, 128, N2), np.float32)
    for u in range(4):
        yrep[:, 32 * u:32 * u + 5] = ytq[:, :, :, u].reshape(B, 5, 128 * NQUAD)
        qrep[:, 32 * u:32 * u + 5] = qtil

    # vt[b][p, 4*i + c] = (c < 3 ? flow1[b, c, i*128 + p] : 1)
    vtil = np.empty((B, 128, NT1, 4), np.float32)
    vtil[:, :, :, 3] = 1.0
    vtil[:, :, :, 0:3] = flow1.reshape(B, 3, NT1, 128).transpose(0, 3, 2, 1)
    vtil = vtil.reshape(B, 128, 4 * NT1)

    yrep = yrep.astype(bf16)
    qrep = qrep.astype(bf16)
    vtil = vtil.astype(bf16)

    in_maps = []
    for k in range(N_CORES):
        b, q = divmod(k, 4)
        js = slice(q * CHUNK, (q + 1) * CHUNK)
        in_maps.append(
            {
                "qt": np.ascontiguousarray(qrep[b][:, js]),
                "yt": yrep[b],
                "vt": vtil[b],
                "x2": np.ascontiguousarray(xyz2[b][:, js]),
            }
        )

    trace = bool(int(os.environ.get("PW_TRACE", "0")))
    if trace:
        try:
            _install_ntff_shim()
        except Exception as e:  # profiling is best-effort
            print(f"ntff shim failed: {e}", file=sys.stderr)

    nc = _build_nc(inv_scale2)
    res = run_bass_kernel_spmd(
        nc,
        in_maps,
        core_ids=list(range(N_CORES)),
        trace=trace,
    )
    LAST_RESULTS = res

    out = np.empty((B, C, N2), np.float32)
    for k in range(N_CORES):
        b, q = divmod(k, 4)
        out[b][:, q * CHUNK:(q + 1) * CHUNK] = res.results[k]["out"]
    return out


# revision 13
# speedup vs baseline: 1.4129x; 1.4129x over previous
"""PointWarping2 (Gaussian-kernel Nadaraya-Watson flow regression) on 8 TRN2 cores.

Math (per batch b):
    y      = xyz1 + flow1                     # warped sources  [N1, 3]
    d2     = ||x2_n - y_m||^2                 # [N2, N1]
    K      = exp(-d2 / scale^2)
    flow2  = (K @ [f1|1]) ratios              # Nadaraya-Watson
    out    = x2 - flow2                       # [3, N2]

Device strategy (per core; 8 cores = 2 batches x 4 query-chunks of 2048):
    T[m, n] = exp(-d2[n, m]) computed in "source-major" layout so the second
    matmul (contraction over sources) consumes T tiles directly.

    Loop: j over 512-query chunks, t over triads of 3 source tiles.
      mm1 (x3, row-packed via tile_position=(32u, 0), K=5):
           S[:, 512u:512u+512] = Yrep[32u:32u+5, triad t].T @ Qrep[32u:32u+5, j]
           with Y rows [y0, y1, y2, |y|^2, 1], Q rows [-2x, 1, |x|^2]
           => S = d2 exactly (f32 PSUM, 3 banks, double buffered)
      ACT: t[128, 1536] = exp(-S / scale^2)   (bf16, SBUF) — one call per triad
      mm2 (x3, K=128, M=97): acc[97, 512](PSUM) += Vexp[:, tile].T @ t-slice
           accumulated over all 64 source tiles; Vexp columns [f0@0, f1@32,
           f2@64, 1@96] put num/den rows on 32-aligned partitions (compute
           APs may only start at partitions 0/32/64/96).
    epilogue per j (DVE): flow2 = num * recip(den); out = x2 - flow2.
"""

import os
import sys

import numpy as np

sys.path.insert(0, "/opt/trn_rl_repo")

import ml_dtypes

import concourse.bass as bass
import concourse.mybir as mybir
import concourse.tile as tile
from concourse import bacc
from concourse.bass_utils import run_bass_kernel_spmd

B, C, N1, N2 = 2, 3, 8192, 8192
INITIAL_RADIUS = 1.0
N_CORES = 8
CHUNK = N2 // 4          # queries per core (2 batches x 4 chunks)
JW = 512                 # n2 width per pass
NT1 = N1 // 128          # 64 source tiles of 128
GROUP = 3                # source tiles per triad / exp() call (3 psum banks)
NTRI = (NT1 + GROUP - 1) // GROUP
TRIADS = [GROUP] * (NT1 // GROUP) + ([NT1 % GROUP] if NT1 % GROUP else [])

LAST_RESULTS = None      # BassKernelResults of the most recent run (for test.py)


def _install_ntff_shim():
    """Register the axon NTFF profiling hook under antenv.axon_hooks (the
    agent image's antenv lacks that submodule) so run_bass_kernel_spmd's
    trace=True path can capture real HW timing. Trace-mode only."""
    import types

    import antenv

    if "antenv.axon_hooks" in sys.modules:
        return
    from trn_agent_boot.trn_boot import _ntff_profile_via_ctypes

    hook = _ntff_profile_via_ctypes("/opt/axon/libaxon_pjrt.so")
    mod = types.ModuleType("antenv.axon_hooks")
    mod._hook = hook
    mod.get_axon_ntff_profile_hook = lambda: mod._hook
    mod.set_axon_ntff_profile_hook = lambda h: setattr(mod, "_hook", h)
    sys.modules["antenv.axon_hooks"] = mod
    antenv.axon_hooks = mod

    # No S3 in this container: stub the artifact upload the trace path does.
    import concourse.bass_utils as bu

    bu.upload_artifacts = lambda tmpdir: tmpdir


def _build_nc(inv_scale2: float) -> bass.Bass:
    nc = bacc.Bacc("TRN2", target_bir_lowering=False, debug=False)
    bf16 = mybir.dt.bfloat16
    f32 = mybir.dt.float32

    qt_d = nc.dram_tensor("qt", [128, CHUNK], bf16, kind="ExternalInput")
    yt_d = nc.dram_tensor("yt", [128, 128 * NTRI], bf16, kind="ExternalInput")
    vx_d = nc.dram_tensor("vx", [128, 97 * NT1], bf16, kind="ExternalInput")
    x2_d = nc.dram_tensor("x2", [3, CHUNK], f32, kind="ExternalInput")
    out_d = nc.dram_tensor("out", [3, CHUNK], f32, kind="ExternalOutput")

    with tile.TileContext(nc) as tc:
        with (
            tc.tile_pool(name="const", bufs=1) as cpool,
            tc.tile_pool(name="work", bufs=3) as wpool,
            tc.tile_pool(name="spsum", bufs=2, space="PSUM") as spool,
            tc.tile_pool(name="apsum", bufs=2, space="PSUM") as apool,
        ):
            vexp = cpool.tile([128, 97 * NT1], bf16)
            # chunked so early mm2 deps resolve before the whole V arrives
            for h in range(4):
                w = 97 * NT1 // 4
                nc.sync.dma_start(vexp[:, h * w:(h + 1) * w], vx_d[:, h * w:(h + 1) * w])
            qt = cpool.tile([128, CHUNK], bf16)
            yt = cpool.tile([128, 128 * NTRI], bf16)
            # x2 rows as separate base-0 tiles: compute-engine APs may only
            # start at partition 0/32/64/96, so [3, N] row slices are illegal.
            x2r = [cpool.tile([1, CHUNK], f32, tag=f"x2_{c}", name=f"x2_{c}") for c in range(3)]
            nc.sync.dma_start(qt[:], qt_d[:])
            nc.sync.dma_start(yt[:], yt_d[:])
            for c in range(3):
                nc.sync.dma_start(x2r[c][:], x2_d[c:c + 1, :])

            for j in range(CHUNK // JW):
                js = slice(j * JW, (j + 1) * JW)
                acc = apool.tile([97, JW], f32, tag="acc")
                prev = None

                def emit_mm2(prev_t, prev_ti, acc=acc):
                    base = prev_ti * GROUP
                    for u in range(TRIADS[prev_ti]):
                        m = base + u
                        nc.tensor.matmul(
                            acc[:],
                            vexp[:, 97 * m:97 * m + 97],
                            prev_t[:, u * JW:(u + 1) * JW],
                            start=(m == 0),
                            stop=(m == NT1 - 1),
                        )

                for ti, gsz in enumerate(TRIADS):
                    s = spool.tile([128, GROUP * JW], f32, tag="s")
                    for u in range(gsz):
                        nc.tensor.matmul(
                            s[:, u * JW:(u + 1) * JW],
                            yt[32 * u:32 * u + 5, 128 * ti:128 * (ti + 1)],
                            qt[32 * u:32 * u + 5, js],
                            start=True,
                            stop=True,
                            tile_position=(32 * u, 0),
                        )
                    # mm2 of the previous triad is emitted after this triad's
                    # mm1 so the in-order PE queue never blocks mm1(t+1)
                    # behind mm2(t)'s wait on ACT(t).
                    if prev is not None:
                        emit_mm2(*prev)
                    t = wpool.tile([128, GROUP * JW], bf16, tag="t")
                    nc.scalar.activation(
                        t[:, :gsz * JW],
                        s[:, :gsz * JW],
                        mybir.ActivationFunctionType.Exp,
                        scale=-float(inv_scale2),
                    )
                    prev = (t, ti)
                emit_mm2(*prev)

                # Epilogue on DVE: rows live at 32-aligned partitions; acc is
                # PSUM so mixed-base two-input ops are legal.
                rec = wpool.tile([1, JW], f32, tag="rec")
                scr = wpool.tile([1, JW], f32, tag="scr")
                nc.vector.reciprocal_approx_accurate(rec[:], acc[96:97, :], scr[:])
                for c in range(3):
                    ob = wpool.tile([1, JW], f32, tag=f"ob_{c}", name=f"ob_{c}")
                    nc.vector.tensor_mul(ob[:], acc[32 * c:32 * c + 1, :], rec[:])
                    nc.vector.tensor_sub(ob[:], x2r[c][:, js], ob[:])
                    nc.sync.dma_start(out_d[c:c + 1, js], ob[:])

    nc.compile()
    return nc


def _host_prep(xyz1, xyz2, flow1):
    bf16 = ml_dtypes.bfloat16
    y = xyz1 + flow1                                  # [B, 3, N1]
    ytil = np.empty((B, 5, N1), np.float32)
    ytil[:, 0:3] = y
    ytil[:, 3] = np.sum(y * y, axis=1)
    ytil[:, 4] = 1.0
    qtil = np.empty((B, 5, N2), np.float32)
    qtil[:, 0:3] = -2.0 * xyz2
    qtil[:, 3] = 1.0
    qtil[:, 4] = np.sum(xyz2 * xyz2, axis=1)

    # Row-replicated layouts for tile_position row-packing: strip u (partition
    # base 32u) of triad column ti holds source tile GROUP*ti+u; queries
    # replicated on the three strips.
    yrep = np.zeros((B, 128, 128 * NTRI), np.float32)
    qrep = np.zeros((B, 128, N2), np.float32)
    yt_tiles = ytil.reshape(B, 5, NT1, 128)           # [B, r, m, p]
    for u in range(GROUP):
        qrep[:, 32 * u:32 * u + 5] = qtil
        for ti in range(NTRI):
            m = GROUP * ti + u
            if m < NT1:
                yrep[:, 32 * u:32 * u + 5, 128 * ti:128 * (ti + 1)] = yt_tiles[:, :, m]

    # Vexp[b][p, 97*m + 32*c] = (c < 3 ? flow1[b, c, m*128 + p] : 1)
    vexp = np.zeros((B, 128, 97 * NT1), np.float32)
    f_t = flow1.reshape(B, 3, NT1, 128)               # [B, c, m, p]
    for c in range(3):
        vexp[:, :, 32 * c::97] = f_t[:, c].transpose(0, 2, 1)
    vexp[:, :, 96::97] = 1.0

    return yrep.astype(bf16), qrep.astype(bf16), vexp.astype(bf16)


def kernel(xyz1, xyz2, flow1, resol_factor):
    global LAST_RESULTS
    xyz1 = np.asarray(xyz1, dtype=np.float32)
    xyz2 = np.asarray(xyz2, dtype=np.float32)
    flow1 = np.asarray(flow1, dtype=np.float32)
    scale = INITIAL_RADIUS * float(np.asarray(resol_factor))
    inv_scale2 = 1.0 / (scale * scale)

    yrep, qrep, vexp = _host_prep(xyz1, xyz2, flow1)

    in_maps = []
    for k in range(N_CORES):
        b, q = divmod(k, 4)
        js = slice(q * CHUNK, (q + 1) * CHUNK)
        in_maps.append(
            {
                "qt": np.ascontiguousarray(qrep[b][:, js]),
                "yt": yrep[b],
                "vx": vexp[b],
                "x2": np.ascontiguousarray(xyz2[b][:, js]),
            }
        )

    trace = bool(int(os.environ.get("PW_TRACE", "0")))
    if trace:
        try:
            _install_ntff_shim()
        except Exception as e:  # profiling is best-effort
            print(f"ntff shim failed: {e}", file=sys.stderr)

    nc = _build_nc(inv_scale2)
    res = run_bass_kernel_spmd(
        nc,
        in_maps,
        core_ids=list(range(N_CORES)),
        trace=trace,
    )
    LAST_RESULTS = res

    out = np.empty((B, C, N2), np.float32)
    for k in range(N_CORES):
        b, q = divmod(k, 4)
        out[b][:, q * CHUNK:(q + 1) * CHUNK] = res.results[k]["out"]
    return out
